# revision 1
# baseline (speedup 1.0000x reference)
"""GAU (Gated Attention Unit) Trainium2 kernel.

Full inputs in, full outputs out.  Sharding: data-parallel over batch
(4 batches x 2 cores); within a batch pair each core owns half the
sequence (2048 query rows) and computes k/v for all 4096 rows locally
(no collectives).  Per-core inputs are reordered own-rows-first so the
SPMD program uses uniform addressing; attention is permutation
invariant over the key axis so the reorder is harmless.

Device pipeline per core:
  LN stats (row-major) -> normalize+cast bf16 -> DMA-transpose to
  feature-major normed^T -> Z^T = Wqk^T@normed^T -> q^T,k^T
  -> v (row-major) and gate^T via W_hidden matmuls
  -> per 512-row i-block: simT = k^T-stationary matmuls, A^T=relu(sim)^2,
     V^T accumulation with v-stationary matmuls, Vg^T = V^T*gate^T,
     out = Vg^T-stationary @ W_out + residual.

norm_scale/norm_bias are folded into W_hidden/W_qk on the host
(layernorm affine commutes into the following linear layers), and the
1/seq_len on q is folded into gamma[0]/beta[0].
"""

import os
import sys

import numpy as np

for _p in ("/opt/trn_rl_repo", "/root/.axon_site/_ro/trn_rl_repo"):
    if os.path.isdir(_p) and _p not in sys.path:
        sys.path.insert(0, _p)
        break

import ml_dtypes  # noqa: E402

import concourse.bass as bass  # noqa: E402
import concourse.tile as tile  # noqa: E402
from concourse import mybir  # noqa: E402

AF = mybir.ActivationFunctionType
ALU = mybir.AluOpType
AX = mybir.AxisListType
DT = mybir.dt
BF16 = ml_dtypes.bfloat16

B, S, D = 4, 4096, 512
H = 1024          # v width == gate width
QK = 128
SO = S // 2       # own rows per core
NCORES = 8
EPS = 1e-5

RT = 32           # row tiles of 128 over S
GT = 4            # LN row-tile groups of 8
FC = D // 128     # feature chunks (4)
HC = H // 128     # hidden chunks (8)
IB = 512          # attention i-block
NBLK = SO // IB   # 4
JT = S // 128     # key chunks (32)


def _build(flags, split=True):
    """Build the SPMD Bass program.  flags = (use_bv, use_bout)."""
    use_bv, use_bout = flags
    nc = bass.Bass()

    xa_d = nc.declare_dram_parameter("xa", [S, D], DT.float32, isOutput=False)
    whid_d = nc.declare_dram_parameter("whid", [D, 2 * H], DT.bfloat16, isOutput=False)
    wqk_d = nc.declare_dram_parameter("wqk", [D, QK], DT.bfloat16, isOutput=False)
    wout_d = nc.declare_dram_parameter("wout", [H, D], DT.bfloat16, isOutput=False)
    bqk_d = nc.declare_dram_parameter("bqk", [QK], DT.float32, isOutput=False)
    bg_d = nc.declare_dram_parameter("bg", [H], DT.float32, isOutput=False)
    g0s_d = nc.declare_dram_parameter("g0s", [QK], DT.float32, isOutput=False)
    be0s_d = nc.declare_dram_parameter("be0s", [QK], DT.float32, isOutput=False)
    g1_d = nc.declare_dram_parameter("g1", [QK], DT.float32, isOutput=False)
    be1_d = nc.declare_dram_parameter("be1", [QK], DT.float32, isOutput=False)
    if use_bv:
        bv_d = nc.declare_dram_parameter("bv", [H], DT.float32, isOutput=False)
    if use_bout:
        bout_d = nc.declare_dram_parameter("bout", [D], DT.float32, isOutput=False)
    out_d = nc.declare_dram_parameter("out", [SO, D], DT.float32, isOutput=True)

    with tile.TileContext(nc) as tc:
        with tc.tile_pool(name="persist", bufs=1) as pp:
            kT = pp.tile([128, S], DT.bfloat16)
            qT = pp.tile([128, SO], DT.bfloat16)
            v_sb = pp.tile([128, RT, H], DT.bfloat16)
            gT = pp.tile([128, HC, SO], DT.bfloat16)
            wout_sb = pp.tile([128, HC, D], DT.bfloat16)
            bqk_sb = pp.tile([128, 1], DT.float32)
            bg_sb = pp.tile([128, HC], DT.float32)
            g0s_sb = pp.tile([128, 1], DT.float32)
            be0s_sb = pp.tile([128, 1], DT.float32)
            g1_sb = pp.tile([128, 1], DT.float32)
            be1_sb = pp.tile([128, 1], DT.float32)
            nc.sync.dma_start(wout_sb[:], wout_d[:].rearrange("(c p) d -> p c d", p=128))
            nc.sync.dma_start(bqk_sb[:], bqk_d[:].unsqueeze(1))
            nc.sync.dma_start(bg_sb[:], bg_d[:].rearrange("(c p) -> p c", p=128))
            nc.sync.dma_start(g0s_sb[:], g0s_d[:].unsqueeze(1))
            nc.sync.dma_start(be0s_sb[:], be0s_d[:].unsqueeze(1))
            nc.sync.dma_start(g1_sb[:], g1_d[:].unsqueeze(1))
            nc.sync.dma_start(be1_sb[:], be1_d[:].unsqueeze(1))
            if use_bv:
                bv_rep = pp.tile([128, H], DT.float32)
                nc.sync.dma_start(
                    bv_rep[:], bv_d[:].unsqueeze(0).partition_broadcast(128)
                )
            if use_bout:
                bout_rep = pp.tile([128, D], DT.float32)
                nc.sync.dma_start(
                    bout_rep[:], bout_d[:].unsqueeze(0).partition_broadcast(128)
                )

            # ---------- phase 1: layernorm + transpose ----------
            with tc.tile_pool(name="pre", bufs=1) as prep:
                normT = prep.tile([128, FC, S], DT.bfloat16)
                whid_sb = prep.tile([128, FC, 2 * H], DT.bfloat16)
                wqk_sb = prep.tile([128, FC, QK], DT.bfloat16)
                nc.sync.dma_start(
                    whid_sb[:], whid_d[:].rearrange("(f p) h -> p f h", p=128)
                )
                nc.sync.dma_start(
                    wqk_sb[:], wqk_d[:].rearrange("(f p) q -> p f q", p=128)
                )

                with (
                    tc.tile_pool(name="lnx", bufs=2) as lnx,
                    tc.tile_pool(name="lns", bufs=2) as lns,
                    tc.tile_pool(name="lnst", bufs=2) as lnst,
                    tc.tile_pool(name="lnb", bufs=3) as lnb,
                ):
                    for g in range(GT):
                        xg = lnx.tile([128, 8, D], DT.float32)
                        nc.sync.dma_start(
                            xg[:],
                            xa_d[g * 1024 : (g + 1) * 1024, :].rearrange(
                                "(t p) d -> p t d", p=128
                            ),
                        )
                        s1 = lnst.tile([128, 8], DT.float32, tag="s1")
                        ssq = lnst.tile([128, 8], DT.float32, tag="ssq")
                        for t in range(8):
                            sq = lns.tile([128, D], DT.float32, tag="sq")
                            nc.vector.tensor_reduce(
                                s1[:, t : t + 1], xg[:, t, :], AX.X, ALU.add
                            )
                            nc.scalar.activation(
                                sq[:], xg[:, t, :], AF.Square,
                                accum_out=ssq[:, t : t + 1],
                            )
                        mu = lnst.tile([128, 8], DT.float32, tag="mu")
                        nmusq = lnst.tile([128, 8], DT.float32, tag="nmusq")
                        var = lnst.tile([128, 8], DT.float32, tag="var")
                        veps = lnst.tile([128, 8], DT.float32, tag="veps")
                        rvar = lnst.tile([128, 8], DT.float32, tag="rvar")
                        rsv = lnst.tile([128, 8], DT.float32, tag="rsv")
                        nmu = lnst.tile([128, 8], DT.float32, tag="nmu")
                        nc.scalar.mul(mu[:], s1[:], 1.0 / D)
                        nc.vector.scalar_tensor_tensor(
                            nmusq[:], mu[:], -1.0, mu[:], ALU.mult, ALU.mult
                        )
                        nc.vector.scalar_tensor_tensor(
                            var[:], ssq[:], 1.0 / D, nmusq[:], ALU.mult, ALU.add
                        )
                        nc.vector.tensor_scalar_add(veps[:], var[:], EPS)
                        nc.vector.reciprocal(rvar[:], veps[:])
                        nc.scalar.activation(rsv[:], rvar[:], AF.Sqrt)
                        nc.vector.scalar_tensor_tensor(
                            nmu[:], mu[:], -1.0, rsv[:], ALU.mult, ALU.mult
                        )
                        for t in range(8):
                            r = g * 8 + t
                            sb = lnb.tile([128, D], DT.bfloat16, tag="sb")
                            nc.vector.tensor_scalar(
                                sb[:], xg[:, t, :],
                                rsv[:, t : t + 1], nmu[:, t : t + 1],
                                ALU.mult, ALU.add,
                            )
                            for f in range(FC):
                                nc.scalar.dma_start(
                                    normT[:, f, r * 128 : (r + 1) * 128],
                                    sb[:, f * 128 : (f + 1) * 128],
                                    transpose=True,
                                )

                # ---------- phase 2: Z -> q,k ----------
                with (
                    tc.tile_pool(name="zp", bufs=2, space="PSUM") as zp,
                    tc.tile_pool(name="zs", bufs=2) as zs,
                ):
                    for n in range(S // 512):
                        ps = zp.tile([128, 512], DT.float32)
                        for f in range(FC):
                            nc.tensor.matmul(
                                ps[:], wqk_sb[:, f, :],
                                normT[:, f, n * 512 : (n + 1) * 512],
                                start=(f == 0), stop=(f == FC - 1),
                            )
                        sg = zs.tile([128, 512], DT.float32, tag="sg")
                        sil = zs.tile([128, 512], DT.float32, tag="sil")
                        nc.scalar.activation(
                            sg[:], ps[:], AF.Sigmoid, bias=bqk_sb[:, 0:1]
                        )
                        nc.vector.scalar_tensor_tensor(
                            sil[:], ps[:], bqk_sb[:, 0:1], sg[:],
                            ALU.add, ALU.mult,
                        )
                        nc.vector.tensor_scalar(
                            kT[:, n * 512 : (n + 1) * 512], sil[:],
                            g1_sb[:, 0:1], be1_sb[:, 0:1], ALU.mult, ALU.add,
                        )
                        if n < SO // 512:
                            nc.vector.tensor_scalar(
                                qT[:, n * 512 : (n + 1) * 512], sil[:],
                                g0s_sb[:, 0:1], be0s_sb[:, 0:1], ALU.mult, ALU.add,
                            )

                # ---------- phase 3: v (row-major) ----------
                with (
                    tc.tile_pool(name="vp", bufs=2, space="PSUM") as vp,
                    tc.tile_pool(name="vt", bufs=2) as vt,
                ):
                    for r in range(RT):
                        ps = vp.tile([128, H], DT.float32)
                        for f in range(FC):
                            lhsT = normT[:, f, r * 128 : (r + 1) * 128]
                            nc.tensor.matmul(
                                ps[:, 0:512], lhsT, whid_sb[:, f, 0:512],
                                start=(f == 0), stop=(f == FC - 1),
                            )
                            nc.tensor.matmul(
                                ps[:, 512:1024], lhsT, whid_sb[:, f, 512:1024],
                                start=(f == 0), stop=(f == FC - 1),
                            )
                        vsg = vt.tile([128, H], DT.float32, tag="vsg")
                        if use_bv:
                            tmp = vt.tile([128, H], DT.float32, tag="tmp")
                            nc.vector.tensor_add(tmp[:], ps[:], bv_rep[:])
                            nc.scalar.activation(vsg[:], tmp[:], AF.Sigmoid)
                            nc.vector.tensor_mul(v_sb[:, r, :], tmp[:], vsg[:])
                        else:
                            nc.scalar.activation(vsg[:], ps[:], AF.Sigmoid)
                            nc.vector.tensor_mul(v_sb[:, r, :], ps[:], vsg[:])

                # ---------- phase 4: gate^T ----------
                with (
                    tc.tile_pool(name="gp", bufs=1, space="PSUM") as gp,
                    tc.tile_pool(name="gs", bufs=2) as gs,
                ):
                    for h in range(HC):
                        ps = gp.tile([128, SO], DT.float32)
                        for f in range(FC):
                            lhsT = whid_sb[:, f, H + h * 128 : H + (h + 1) * 128]
                            for i4 in range(SO // 512):
                                nc.tensor.matmul(
                                    ps[:, i4 * 512 : (i4 + 1) * 512], lhsT,
                                    normT[:, f, i4 * 512 : (i4 + 1) * 512],
                                    start=(f == 0), stop=(f == FC - 1),
                                )
                        for i4 in range(SO // 512):
                            gsg = gs.tile([128, 512], DT.float32, tag="gsg")
                            nc.scalar.activation(
                                gsg[:], ps[:, i4 * 512 : (i4 + 1) * 512],
                                AF.Sigmoid, bias=bg_sb[:, h : h + 1],
                            )
                            nc.vector.scalar_tensor_tensor(
                                gT[:, h, i4 * 512 : (i4 + 1) * 512],
                                ps[:, i4 * 512 : (i4 + 1) * 512],
                                bg_sb[:, h : h + 1], gsg[:],
                                ALU.add, ALU.mult,
                            )

            # ---------- phase 5: attention + output ----------
            with (
                tc.tile_pool(name="attnA", bufs=1) as pa,
                tc.tile_pool(name="attnR", bufs=3) as pr,
                tc.tile_pool(name="attnVg", bufs=2) as pvg,
                tc.tile_pool(name="attnX", bufs=2) as px,
                tc.tile_pool(name="attnO", bufs=2) as po_sb,
                tc.tile_pool(name="psim", bufs=2, space="PSUM") as psim,
                tc.tile_pool(name="pV", bufs=1, space="PSUM") as pV,
                tc.tile_pool(name="pout", bufs=2, space="PSUM") as pout,
            ):
                A_sb = pa.tile([128, JT, IB], DT.bfloat16)
                for blk in range(NBLK):
                    i0 = blk * IB
                    # simT -> A^T
                    for j in range(JT):
                        ps = psim.tile([128, IB], DT.float32)
                        nc.tensor.matmul(
                            ps[:], kT[:, j * 128 : (j + 1) * 128],
                            qT[:, i0 : i0 + IB], start=True, stop=True,
                        )
                        rt = pr.tile([128, IB], DT.bfloat16)
                        nc.scalar.activation(rt[:], ps[:], AF.Relu)
                        nc.vector.tensor_mul(A_sb[:, j, :], rt[:], rt[:])
                    # V^T accumulation, two h-halves of 512
                    vg = pvg.tile([128, HC, IB], DT.bfloat16)
                    for hh in range(2):
                        pvt = [
                            pV.tile(
                                [128, IB], DT.float32,
                                name=f"pvt{q}", tag=f"pvt{q}",
                            )
                            for q in range(4)
                        ]
                        for j in range(JT):
                            for hq in range(4):
                                h = hh * 4 + hq
                                nc.tensor.matmul(
                                    pvt[hq][:],
                                    v_sb[:, j, h * 128 : (h + 1) * 128],
                                    A_sb[:, j, :],
                                    start=(j == 0), stop=(j == JT - 1),
                                )
                        for hq in range(4):
                            h = hh * 4 + hq
                            nc.vector.tensor_tensor(
                                vg[:, h, :], pvt[hq][:],
                                gT[:, h, i0 : i0 + IB], ALU.mult,
                            )
                    # out = Vg^T-stationary @ W_out + x (+ b_out)
                    for ic in range(IB // 128):
                        r0 = i0 + ic * 128
                        ps = pout.tile([128, D], DT.float32)
                        for h in range(HC):
                            nc.tensor.matmul(
                                ps[:], vg[:, h, ic * 128 : (ic + 1) * 128],
                                wout_sb[:, h, :],
                                start=(h == 0), stop=(h == HC - 1),
                            )
                        xo = px.tile([128, D], DT.float32)
                        nc.sync.dma_start(xo[:], xa_d[r0 : r0 + 128, :])
                        ot = po_sb.tile([128, D], DT.float32)
                        nc.vector.tensor_add(ot[:], ps[:], xo[:])
                        if use_bout:
                            nc.vector.tensor_add(ot[:], ot[:], bout_rep[:])
                        nc.sync.dma_start(out_d[r0 : r0 + 128, :], ot[:])

    nc.finalize()
    if split:
        _split_waits(nc)
    return nc


# The walrus build in this container supports very few semaphore waits per
# hardware instruction (an Activation with 2 waits or a Drain with 3 fails
# codegen with "Too many sync wait commands").  Tile freely emits
# multi-wait instructions, so hoist all but one wait of each instruction
# into dedicated single-wait EventSemaphore instructions placed immediately
# before it on the same engine queue — semantically identical, just split.
_MAX_WAITS = 1


def _split_waits(nc):
    n_new = 0
    for fn in nc.m.functions:
        for bb in fn.blocks:
            out = []
            changed = False
            for inst in bb.instructions:
                si = inst.sync_info
                if si is not None and len(si.on_wait) > _MAX_WAITS:
                    waits = list(si.on_wait)
                    for w in waits[:-_MAX_WAITS]:
                        es = mybir.InstEventSemaphore(
                            name=f"{inst.name}-w{n_new}", ins=[], outs=[],
                            engine=inst.engine,
                        )
                        es.sync_info = mybir.SyncInfo(on_wait=[w], on_update=[])
                        out.append(es)
                        n_new += 1
                    inst.sync_info = mybir.SyncInfo(
                        on_wait=waits[-_MAX_WAITS:],
                        on_update=list(si.on_update),
                    )
                    changed = True
                out.append(inst)
            if changed:
                bb.instructions = out
    return n_new


_PROGRAM_CACHE = {}


def _get_program(flags):
    if flags not in _PROGRAM_CACHE:
        _PROGRAM_CACHE[flags] = _build(flags)
    return _PROGRAM_CACHE[flags]


def _prep(inputs):
    x = np.ascontiguousarray(np.asarray(inputs["x"], dtype=np.float32))
    scale = np.asarray(inputs["norm_scale"], dtype=np.float32)
    bias = np.asarray(inputs["norm_bias"], dtype=np.float32)
    Wh = np.asarray(inputs["W_hidden"], dtype=np.float32)
    bh = np.asarray(inputs["b_hidden"], dtype=np.float32)
    Wq = np.asarray(inputs["W_qk"], dtype=np.float32)
    bq = np.asarray(inputs["b_qk"], dtype=np.float32)
    gamma = np.asarray(inputs["gamma"], dtype=np.float32)
    beta = np.asarray(inputs["beta"], dtype=np.float32)
    Wo = np.asarray(inputs["W_out"], dtype=np.float32)
    bo = np.asarray(inputs["b_out"], dtype=np.float32)

    # Fold layernorm affine into the following linears.
    Whf = scale[:, None] * Wh
    bhf = bias @ Wh + bh
    Wqf = scale[:, None] * Wq
    bqf = bias @ Wq + bq

    bv = bhf[:H]
    bg = bhf[H:]
    use_bv = bool(np.any(bv != 0.0))
    use_bout = bool(np.any(bo != 0.0))

    common = {
        "whid": Whf.astype(BF16),
        "wqk": Wqf.astype(BF16),
        "wout": Wo.astype(BF16),
        "bqk": np.ascontiguousarray(bqf),
        "bg": np.ascontiguousarray(bg),
        "g0s": np.ascontiguousarray(gamma[0] / S),
        "be0s": np.ascontiguousarray(beta[0] / S),
        "g1": np.ascontiguousarray(gamma[1]),
        "be1": np.ascontiguousarray(beta[1]),
    }
    if use_bv:
        common["bv"] = np.ascontiguousarray(bv)
    if use_bout:
        common["bout"] = np.ascontiguousarray(bo)

    in_maps = []
    for c in range(NCORES):
        b, hlf = divmod(c, 2)
        own = x[b, hlf * SO : (hlf + 1) * SO]
        oth = x[b, (1 - hlf) * SO : (2 - hlf) * SO]
        xa = np.ascontiguousarray(np.concatenate([own, oth], axis=0))
        in_maps.append({**common, "xa": xa})
    return (use_bv, use_bout), in_maps


def run_spmd(in_maps, flags, **kw):
    from concourse.bass_utils import run_bass_kernel_spmd

    nc = _get_program(flags)
    return run_bass_kernel_spmd(nc, in_maps, list(range(NCORES)), **kw)


def kernel(**inputs):
    flags, in_maps = _prep(inputs)
    res = run_spmd(in_maps, flags)
    out = np.empty((B, S, D), dtype=np.float32)
    for c in range(NCORES):
        b, hlf = divmod(c, 2)
        out[b, hlf * SO : (hlf + 1) * SO] = res.results[c]["out"]
    return out

